# revision 29
# baseline (speedup 1.0000x reference)
"""Trainium2 Bass kernel for nn_Entropy (KDE local-entropy via histogram binning).

Contract: kernel(**inputs) takes the FULL input x (2,2,1,80,80) fp32 and
returns the FULL output (2,2,80,80) fp32, sharding internally across 8
NeuronCores (core = batch*2 + row-half of the 74x74 patch grid).

Algorithm (per core, one 47x80 input strip -> 37x74 entropy block):
  1. unsharp preprocessing (5x5 blur via PE banded matmul + free-axis tree
     adds, exact-tie-aware rounding, IEEE-reciprocal division) -> integer
     "division" image in [0,255].
  2. 128-bin KDE entropy: division values are merged pairwise (bin id =
     1536 + floor(D/2), produced by the final stage-A op via an fp16 RNE
     write), and the 128x128 kernel matrix is the frequency-weighted
     average of the 256x256 Gaussian over each bin pair (per-image value
     frequencies computed on host from the same preprocessing chain;
     introduces ~8.7e-3 rel err vs the 2e-2 budget, halving all
     downstream work vs 256 bins).
  3. h: fp16 one-hot (bins on partitions, tensor_scalar is_equal in the
     DVE 4x_2p mode) box-summed with shifted-add trees (7 = 4+2+1), all
     on the DVE (GpSimd helpers lose via shared-SBUF-port contention).
     The image is broadcast to 128 partitions by K=1 PE matmuls chunked
     through PSUM with ACT copies; the early tree levels are emitted in
     chunk-gated row pieces so they overlap the broadcast, and the tree
     is split into two row bands so band A's stage C overlaps band B.
  4. stage C per chunk: G = K' @ h (PE), lp = Ln(G*s + eps) (ACT),
     m = h.*lp (DVE), e-rows += wcol_k^T m (PE, accumulated in two PSUM
     banks A/B so band A's output drains early). Final -(1/49) on host.
"""
import os
import sys

import numpy as np

for _p in ("/opt/trn_rl_repo", "/root/.axon_site/_ro/trn_rl_repo"):
    if os.path.isdir(_p) and _p not in sys.path:
        sys.path.insert(0, _p)

import concourse.bass as bass
import concourse.bacc as bacc
import concourse.tile as tile
from concourse import mybir
from concourse.bass_utils import run_bass_kernel_spmd

dt = mybir.dt
Alu = mybir.AluOpType
Act = mybir.ActivationFunctionType
f32 = np.float32

R = 7
BW = 2.5
L = R * R  # 49
NORM = f32((2.0 * np.pi * BW * BW) ** 0.5)  # C=1 -> exponent 1/2
LN_SCALE = float(f32(1.0 / (L * NORM)))
INV25 = float(f32(1.0) / f32(25.0))

# geometry
HP = 74          # patch grid cols (80 - 7 + 1)
ROWS = 43        # division-image rows needed per core (37 patch rows + 6)
PR = 37          # patch rows per core
NPIX = ROWS * 80         # 3440
NP_ = PR * HP            # 2738 patches per core
CHUNK = 512
BAND = 20        # tree band A patch rows (0..BAND-1); band B = BAND..36
RB = PR - BAND   # 17
SPLIT = BAND * HP  # h columns boundary between the two tree bands

MAGIC = 8388608.0  # fp32 RNE trick: (v + 2^23) - 2^23

_COMPILED = None  # compiled Bacc program (input-independent)


def _division_host(xi):
    """Bit-faithful host replica of the on-device preprocessing for one
    80x80 image; used only to derive per-image bin frequencies for the
    merged kernel matrix."""
    from numpy.lib.stride_tricks import sliding_window_view

    pad = np.pad(xi.astype(f32), ((2, 2), (2, 2)))
    sm = np.round(sliding_window_view(pad, (5, 5)).sum(axis=(2, 3), dtype=np.float64)
                  / 25.0).astype(f32)
    sh = np.round(np.clip(f32(2.5) * xi - f32(1.25) * sm, 0.0, 255.0)).astype(f32)
    return np.round(np.clip(sh * f32(255.0) / (sm + f32(1e-8)), 0.0, 255.0)).astype(f32)


def _merged_kernel(freq):
    """128x128 frequency-weighted merged Gaussian kernel (fp16)."""
    v = np.arange(256, dtype=np.float64)
    Kfull = np.exp(-((v[:, None] - v[None, :]) ** 2) / (2.0 * BW * BW))
    f = freq.astype(np.float64) + 1e-3
    Kp = np.zeros((128, 128))
    for a in range(2):
        for b in range(2):
            Kp += np.outer(f[a::2], f[b::2]) * Kfull[a::2][:, b::2]
    FB = f[0::2] + f[1::2]
    Kp /= np.outer(FB, FB)
    return Kp.astype(np.float16)


def _host_constants(x4):
    """Per-core constant tensors. x4: (4, 80, 80) fp32 full input."""
    consts = []
    for img in range(4):
        dv = _division_host(x4[img])
        freq = np.bincount(dv.astype(np.int32).ravel(), minlength=256)
        kmat = _merged_kernel(freq)

        cf16 = np.zeros((128, 306), np.float16)
        cf16[:, 0:128] = kmat
        cf16[0, 129:257] = 1.0      # ones row (broadcast lhsT)
        for k in range(7):          # wcol_k: ones in column k -> e row k
            cf16[:, 257 + 7 * k + k] = 1.0

        cf32 = np.zeros((128, 88), f32)
        for m in range(ROWS):
            cf32[m: m + 5, m] = 1.0  # b5 banded blur matrix [47, 43]
        cf32[:, 43] = 1536.0 + np.arange(128, dtype=f32)  # bin match values
        for m in range(ROWS):
            cf32[m + 2, 44 + m] = 2.5  # xmid selector: 2.5 * x[row m+2]
        consts.append({"cf16": cf16, "cf32": cf32})
    return consts


def _build_nc():
    nc = bacc.Bacc("TRN2", target_bir_lowering=False, debug=False)

    xs_d = nc.dram_tensor("xs", [47, 80], dt.float32, kind="ExternalInput")
    cf32_d = nc.dram_tensor("cf32", [128, 88], dt.float32, kind="ExternalInput")
    cf16_d = nc.dram_tensor("cf16", [128, 306], dt.float16, kind="ExternalInput")
    ent_d = nc.dram_tensor("ent", [7, 512], dt.float32, kind="ExternalOutput")

    # broadcast/one-hot chunks (forward order: band A consumes low rows first)
    oh_chunks = []
    off = 0
    while off < NPIX:
        cw = min(CHUNK, NPIX - off)
        oh_chunks.append((off, cw))
        off += cw

    # stage-C chunks, aligned to the tree band boundary at SPLIT; the last
    # chunk is kept small to shorten the drain chain
    c_chunks = []
    for lo, hi in ((0, SPLIT), (SPLIT, NP_ - 74)):
        off = lo
        while off < hi:
            cw = min(CHUNK, hi - off)
            c_chunks.append((off, cw))
            off += cw
    c_chunks.append((NP_ - 74, 74))
    NCA = (SPLIT + CHUNK - 1) // CHUNK  # chunks produced by band A

    with tile.TileContext(nc) as tc:
        with (
            tc.tile_pool(name="small", bufs=1) as small,
            tc.tile_pool(name="pre", bufs=1) as pre,
            tc.tile_pool(name="big", bufs=1) as big,
            tc.tile_pool(name="scratch", bufs=1) as scratch,
            tc.tile_pool(name="psum", bufs=3, space="PSUM") as psum,
            tc.tile_pool(name="psum1", bufs=1, space="PSUM") as psum1,
            tc.tile_pool(name="psume", bufs=1, space="PSUM") as psume,
        ):
            # ---------- inputs ----------
            xt = pre.tile([47, 84], dt.float32)
            nc.vector.memset(xt[:], 0.0)
            nc.sync.dma_start(xt[:, 2:82], xs_d[:])
            c32 = small.tile([128, 88], dt.float32)
            nc.scalar.dma_start(c32[:], cf32_d[:])
            c16 = small.tile([128, 306], dt.float16)
            nc.scalar.dma_start(c16[:], cf16_d[:])
            eps_t = small.tile([128, 1], dt.float32)
            nc.vector.memset(eps_t[:], 1e-8)

            b5v = c32[0:47, 0:43]
            binsv = c32[:, 43:44]
            xselv = c32[0:47, 44:87]
            kmatv = c16[:, 0:128]
            onesrow = c16[0:1, 129:257]

            # ---------- stage A: preprocessing -> division [43, 80] ----------
            sv_ps = psum1.tile([ROWS, 84], dt.float32, tag="mps")
            nc.tensor.matmul(sv_ps[:], b5v, xt[:], start=True, stop=True)
            sv = pre.tile([ROWS, 84], dt.float32)
            nc.scalar.copy(sv[:], sv_ps[:])
            sv = pre.tile([ROWS, 84], dt.float32)
            nc.scalar.copy(sv[:], sv_ps[:])
            xm_ps = psum1.tile([ROWS, 84], dt.float32, tag="xps")
            nc.tensor.matmul(xm_ps[:], xselv, xt[:], start=True, stop=True)

            t1 = pre.tile([ROWS, 83], dt.float32)
            nc.vector.tensor_add(t1[:], sv_ps[:, 0:83], sv[:, 1:84])
            t2 = pre.tile([ROWS, 81], dt.float32)
            nc.vector.tensor_add(t2[:], t1[:, 0:81], t1[:, 2:83])
            s25 = pre.tile([ROWS, 80], dt.float32)
            nc.vector.tensor_add(s25[:], t2[:, 0:80], sv_ps[:, 4:84])

            # smooth = RNE(s25/25); magic add/sub in separate instrs, with the
            # -1.25 factor folded into the de-magic step (exact: smooth<=255)
            tt = pre.tile([ROWS, 80], dt.float32)
            nc.vector.tensor_scalar(tt[:], s25[:], INV25, MAGIC, Alu.mult, Alu.add)
            # sharp*255 = 255*RNE(clip(2.5 x - 1.25 smooth, 0, 255))
            sm125 = pre.tile([ROWS, 80], dt.float32)
            nc.vector.tensor_scalar(sm125[:], tt[:], MAGIC, -1.25, Alu.subtract, Alu.mult)
            sp = pre.tile([ROWS, 80], dt.float32)
            nc.vector.tensor_add(sp[:], sm125[:], xm_ps[:, 2:82])
            nc.vector.tensor_scalar(sp[:], sp[:], 0.0, None, Alu.max)
            tt2 = pre.tile([ROWS, 80], dt.float32)
            nc.vector.tensor_scalar(tt2[:], sp[:], 255.0, MAGIC, Alu.min, Alu.add)
            sharp = pre.tile([ROWS, 80], dt.float32)
            nc.vector.tensor_scalar(sharp[:], tt2[:], MAGIC, 255.0, Alu.subtract, Alu.mult)

            # division bin id: 1536 + floor(RNE(min(sharp*255*recip, 255.49))/2)
            denom = pre.tile([ROWS, 80], dt.float32)
            nc.vector.tensor_scalar(denom[:], tt[:], MAGIC, 1e-8, Alu.subtract, Alu.add)
            rr = pre.tile([ROWS, 80], dt.float32)
            nc.vector.reciprocal(rr[:], denom[:])
            vv = pre.tile([ROWS, 80], dt.float32)
            nc.vector.tensor_mul(vv[:], sharp[:], rr[:])
            tt3 = pre.tile([ROWS, 80], dt.float32)
            nc.vector.tensor_scalar(tt3[:], vv[:], 255.49, MAGIC, Alu.min, Alu.add)
            dvt = pre.tile([ROWS, 80], dt.float16)
            nc.vector.tensor_scalar(
                dvt[:], tt3[:], 0.5, 1535.75 - MAGIC * 0.5, Alu.mult, Alu.add
            )

            # ---------- broadcast + merged one-hot ----------
            HOP1 = BAND + 6  # 26 rows cover band A's oh span and chunks 0..3
            dvrow = small.tile([1, NPIX], dt.float16)
            nc.sync.dma_start(dvrow[:, 0: HOP1 * 80], dvt[0:HOP1, :])
            nc.scalar.dma_start(dvrow[:, HOP1 * 80: NPIX], dvt[HOP1:ROWS, :])

            dv_bc = big.tile([128, NPIX], dt.float16, tag="dv_bc")
            oh = big.tile([128, NPIX], dt.float16, tag="oh")
            for ci, (off, cw) in enumerate(oh_chunks):
                bc_ps = psum.tile([128, cw], dt.float32, tag="g_ps", name="bc_ps")
                nc.tensor.matmul(
                    bc_ps[:], onesrow, dvrow[:, off: off + cw],
                    start=True, stop=True,
                )
                if ci == 5:
                    nc.vector.tensor_copy(dv_bc[:, off: off + cw], bc_ps[:])
                else:
                    nc.scalar.copy(dv_bc[:, off: off + cw], bc_ps[:])
            # per-chunk is_equal (4x mode) so the vertical tree can start
            # while later broadcast chunks are still in flight
            for off, cw in oh_chunks:
                nc.vector.tensor_scalar(
                    oh[:, off: off + cw], dv_bc[:, off: off + cw],
                    binsv, None, Alu.is_equal,
                )

            oh3 = oh[:].rearrange("p (r c) -> p r c", r=ROWS, c=80)
            h_f = big.tile([128, NP_], dt.float16, tag="h_f")
            hfv = h_f[:].rearrange("p (r c) -> p r c", r=PR, c=HP)

            # ---------- DVE tree, band A: patch rows 0..BAND-1 ----------
            # v1/v2 computed full-height once (band B reuses them), in row
            # pieces gated on one-hot chunk availability for early overlap
            na1, na2 = ROWS - 2, ROWS - 4  # 41, 39
            v1a = scratch.tile([128, na1 * 80], dt.float16, tag="v1a")
            v1av = v1a[:].rearrange("p (r c) -> p r c", r=na1, c=80)
            for lo, hi in ((0, 9), (9, 17), (17, 24), (24, 30), (30, na1)):
                nc.vector.tensor_add(
                    v1av[:, lo:hi, :], oh3[:, lo:hi, :], oh3[:, lo + 1: hi + 1, :]
                )
            v2a = scratch.tile([128, na2 * 80], dt.float16, tag="v2a")
            v2av = v2a[:].rearrange("p (r c) -> p r c", r=na2, c=80)
            for lo, hi in ((0, 7), (7, 15), (15, 22), (22, 28), (28, na2)):
                nc.vector.tensor_add(
                    v2av[:, lo:hi, :], v1av[:, lo:hi, :], v1av[:, lo + 2: hi + 2, :]
                )
            u2a = scratch.tile([128, BAND * 80], dt.float16, tag="u2a")
            u2av = u2a[:].rearrange("p (r c) -> p r c", r=BAND, c=80)
            for lo, hi in ((0, 13), (13, BAND)):
                nc.vector.tensor_add(
                    u2av[:, lo:hi, :], v2av[:, lo:hi, :], v1av[:, lo + 4: hi + 4, :]
                )
            v7a = scratch.tile([128, BAND * 80], dt.float16, tag="v7a")
            v7av = v7a[:].rearrange("p (r c) -> p r c", r=BAND, c=80)
            for lo, hi in ((0, 13), (13, BAND)):
                nc.vector.tensor_add(
                    v7av[:, lo:hi, :], u2av[:, lo:hi, :], oh3[:, lo + 6: hi + 6, :]
                )

            t1a = scratch.tile([128, BAND * 79], dt.float16, tag="t1a")
            t1av = t1a[:].rearrange("p (r c) -> p r c", r=BAND, c=79)
            nc.vector.tensor_add(t1av, v7av[:, :, 0:79], v7av[:, :, 1:80])
            t2a = scratch.tile([128, BAND * 77], dt.float16, tag="t2a")
            t2av = t2a[:].rearrange("p (r c) -> p r c", r=BAND, c=77)
            nc.vector.tensor_add(t2av, t1av[:, :, 0:77], t1av[:, :, 2:79])
            uha = scratch.tile([128, BAND * HP], dt.float16, tag="uha")
            uhav = uha[:].rearrange("p (r c) -> p r c", r=BAND, c=HP)
            nc.vector.tensor_add(uhav, t2av[:, :, 0:HP], t1av[:, :, 4: 4 + HP])
            nc.vector.tensor_add(hfv[:, 0:BAND, :], uhav, v7av[:, :, 6:80])

            # ---------- DVE tree, band B: patch rows BAND..36 ----------
            u2b = scratch.tile([128, RB * 80], dt.float16, tag="u2a")
            u2bv = u2b[:].rearrange("p (r c) -> p r c", r=RB, c=80)
            nc.vector.tensor_add(
                u2bv, v2av[:, BAND: BAND + RB, :], v1av[:, BAND + 4: BAND + RB + 4, :]
            )
            v7b = scratch.tile([128, RB * 80], dt.float16, tag="v7a")
            v7bv = v7b[:].rearrange("p (r c) -> p r c", r=RB, c=80)
            nc.vector.tensor_add(v7bv, u2bv, oh3[:, BAND + 6: BAND + 6 + RB, :])

            t1b = scratch.tile([128, RB * 79], dt.float16, tag="t1a")
            t1bv = t1b[:].rearrange("p (r c) -> p r c", r=RB, c=79)
            nc.vector.tensor_add(t1bv, v7bv[:, :, 0:79], v7bv[:, :, 1:80])
            t2b = scratch.tile([128, RB * 77], dt.float16, tag="t2a")
            t2bv = t2b[:].rearrange("p (r c) -> p r c", r=RB, c=77)
            nc.vector.tensor_add(t2bv, t1bv[:, :, 0:77], t1bv[:, :, 2:79])
            uhb = scratch.tile([128, RB * HP], dt.float16, tag="uha")
            uhbv = uhb[:].rearrange("p (r c) -> p r c", r=RB, c=HP)
            nc.vector.tensor_add(uhbv, t2bv[:, :, 0:HP], t1bv[:, :, 4: 4 + HP])
            nc.vector.tensor_add(hfv[:, BAND:PR, :], uhbv, v7bv[:, :, 6:80])

            # ---------- stage C: G -> Ln -> h.*lp -> column-sum ----------
            e_psA = psume.tile([4, 512], dt.float32, tag="epsA")
            e_psB = psume.tile([3, 512], dt.float32, tag="epsB")
            ent_sbA = small.tile([4, 512], dt.float32)
            ent_sbB = small.tile([3, 512], dt.float32)
            nb = len(c_chunks) - NCA
            for k, (off, cw) in enumerate(c_chunks[:NCA]):
                hc = h_f[:, off: off + cw]
                g0 = psum.tile([128, cw], dt.float32, tag="g_ps", name=f"g{k}")
                nc.tensor.matmul(g0[:], kmatv, hc, start=True, stop=True)
                lp = scratch.tile([128, cw], dt.float16, tag="lp", name=f"lp{k}", bufs=3)
                nc.scalar.activation(lp[:], g0[:], Act.Ln, bias=eps_t[:], scale=LN_SCALE)
                m0 = scratch.tile([128, cw], dt.float16, tag="m0", name=f"m0{k}", bufs=3)
                nc.vector.tensor_mul(m0[:], hc, lp[:])
                wcol = c16[:, 257 + 7 * k: 257 + 7 * k + 4]
                nc.tensor.matmul(
                    e_psA[0:4, 0:cw], wcol, m0[:],
                    start=(k == 0), stop=(k == NCA - 1),
                )
            for kb, (off, cw) in enumerate(c_chunks[NCA:]):
                k = NCA + kb
                hc = h_f[:, off: off + cw]
                g0 = psum.tile([128, cw], dt.float32, tag="g_ps", name=f"g{k}")
                nc.tensor.matmul(g0[:], kmatv, hc, start=True, stop=True)
                lp = scratch.tile([128, cw], dt.float16, tag="lp", name=f"lp{k}", bufs=3)
                nc.scalar.activation(lp[:], g0[:], Act.Ln, bias=eps_t[:], scale=LN_SCALE)
                m0 = scratch.tile([128, cw], dt.float16, tag="m0", name=f"m0{k}", bufs=3)
                nc.vector.tensor_mul(m0[:], hc, lp[:])
                wcol = c16[:, 257 + 7 * k + 4: 257 + 7 * k + 7]
                nc.tensor.matmul(
                    e_psB[0:3, 0:cw], wcol, m0[:],
                    start=(kb == 0), stop=(kb == nb - 1),
                )
            nc.scalar.copy(ent_sbA[:], e_psA[:])
            nc.sync.dma_start(ent_d[0:4, :], ent_sbA[:])
            nc.scalar.copy(ent_sbB[:], e_psB[:])
            nc.sync.dma_start(ent_d[4:7, :], ent_sbB[:])

    nc.compile()
    return nc


def _get_compiled():
    global _COMPILED
    if _COMPILED is None:
        _COMPILED = _build_nc()
    return _COMPILED


def _run(x, trace=False, **kw):
    """x: (2,2,1,80,80) float32. Returns BassKernelResults."""
    xi = np.ascontiguousarray(np.asarray(x, f32).reshape(4, 80, 80))
    nc = _get_compiled()
    consts = _host_constants(xi)
    in_maps = []
    for core in range(8):
        b, half = core // 2, core % 2
        r0 = half * PR
        strip = np.zeros((47, 80), f32)
        lo, hi = r0 - 2, r0 + 45
        slo, shi = max(lo, 0), min(hi, 80)
        strip[slo - lo: shi - lo] = xi[b, slo:shi]
        m = dict(consts[b])
        m["xs"] = strip
        in_maps.append(m)
    res = run_bass_kernel_spmd(nc, in_maps, list(range(8)), trace=trace, **kw)
    return res


# stage-C chunk layout must match _build_nc
def _c_chunks():
    out = []
    for lo, hi in ((0, SPLIT), (SPLIT, NP_)):
        off = lo
        while off < hi:
            cw = min(CHUNK, hi - off)
            out.append((off, cw))
            off += cw
    return out


def kernel(x):
    res = _run(x)
    out = np.zeros((4, 80, 80), f32)
    pad = R // 2
    chunks = _c_chunks()
    for core in range(8):
        b, half = core // 2, core % 2
        r0 = half * PR
        raw = np.asarray(res.results[core]["ent"], f32)  # [7, 512]
        ent = np.zeros(NP_, f32)
        for k, (off, cw) in enumerate(chunks):
            ent[off: off + cw] = raw[k, 0:cw]
        ent = (ent * f32(-1.0 / L)).reshape(PR, HP)
        out[b, pad + r0: pad + r0 + PR, pad: pad + HP] = ent
    return out.reshape(2, 2, 80, 80)
